# revision 25
# baseline (speedup 1.0000x reference)
"""GCN layer (gnn_message_passing) on 8 Trainium2 NeuronCores.

out = relu(D^-1/2 (A+I) D^-1/2 (x @ W) + b)

Key identity: sum_msgs norm * (xW)[src] = (sum_msgs norm * x[src]) @ W.
Aggregation runs in x-space, so NO full-graph GEMM exists at all: each core
gathers raw x rows for its messages, scatter-accumulates them per 128-target
block on the PE (selection-matrix matmul), and applies W only to its own
aggregated targets (1/8 of the nodes).  Per core:

  - Target blocks of 128 nodes are load-balanced across cores (LPT on
    message count); each core owns 49 blocks (6272 padded target rows).
  - dma_gather pulls x rows (1KB, bf16) for message tiles of 128 straight
    from the x input halves (int16 index range); round-robin over all 4
    SWDGE queues keeps the 4 Q7 descriptor-gen pairs concurrent.  No
    producer dependency: gathers start at t=0.
  - B tiles (B[slot, tau] = dinv[t]*dinv[s] one-hot over the block's 128
    targets) are built on device in bulk: (ramp == tau) * val with
    broadcast tensor_tensor ops from tiny bf16 tau/val tables.
  - Per block: PSUM [128 targets, 512] accumulates B^T @ X_msgs over all
    the block's tiles (lo+hi source halves back-to-back), then
    Z -> PE-transpose -> Z^T @ W -> +bias -> ReLU -> out, software
    pipelined one block deep so the PE never waits on the fixups.

All planning is computed on the host from the integer edge list only; all
floating-point work runs on device.
"""

import ml_dtypes
import numpy as np

import concourse.bacc as bacc
import concourse.bass as bass
import concourse.mybir as mybir
import concourse.tile as tile
from concourse import library_config, masks
from concourse.bass_utils import run_bass_kernel_spmd

BF16 = ml_dtypes.bfloat16
P = 128  # partitions


class Cfg:
    def __init__(self, n, e, di, do, cores, gather_chunk=8, out_batch=4):
        self.n, self.e, self.di, self.do, self.cores = n, e, di, do, cores
        self.tb = P                       # targets per psum block
        self.gather_chunk = gather_chunk  # message tiles per dma_gather call
        self.out_batch = out_batch        # blocks per output DMA
        self.nbt_glob = (n + P - 1) // P          # global target blocks
        self.nbt = (self.nbt_glob + cores - 1) // cores  # blocks per core
        self.nt = self.nbt * P            # padded targets per core
        self.nblocks = (n + P - 1) // P   # node blocks (for x padding)
        self.npad = self.nblocks * P
        self.split_blocks = (self.nblocks + 1) // 2
        self.split = self.split_blocks * P          # lo/hi node boundary
        self.nlo = self.split
        self.nhi_pad = self.npad - self.split
        assert self.nlo <= 32768 and self.nhi_pad <= 32768
        assert di % P == 0
        self.kc = di // P                 # contraction chunks for W


class Plan:
    """Host-side integer/index preprocessing: shared structure (uniform
    across cores, baked into the program) + per-core index/tau/val data."""

    def __init__(self, cfg: Cfg, edge_index: np.ndarray):
        n, C, TB, CH = cfg.n, cfg.cores, cfg.tb, cfg.gather_chunk
        row = np.asarray(edge_index[0], dtype=np.int64)
        col = np.asarray(edge_index[1], dtype=np.int64)
        loop = np.arange(n, dtype=np.int64)
        row = np.concatenate([row, loop])
        col = np.concatenate([col, loop])
        deg = np.bincount(col, minlength=n).astype(np.float64)
        self.dinv = (1.0 / np.sqrt(deg)).astype(np.float32)

        # ---- load-balanced assignment of global target blocks to cores ----
        gblock = col // TB
        gcount = np.bincount(gblock, minlength=cfg.nbt_glob)
        order = np.argsort(-gcount, kind="stable")          # LPT order
        core_load = np.zeros(C, dtype=np.int64)
        core_nblk = np.zeros(C, dtype=np.int64)
        blk_core = np.zeros(cfg.nbt_glob, dtype=np.int64)
        for b in order:
            c = np.argmin(np.where(core_nblk < cfg.nbt, core_load, 1 << 60))
            blk_core[b] = c
            core_load[c] += gcount[b]
            core_nblk[c] += 1
        self.core_blocks = []   # core -> [global block id per slot]
        for c in range(C):
            mine = [b for b in order if blk_core[b] == c]
            mine += [-1] * (cfg.nbt - len(mine))            # pad slots
            self.core_blocks.append(np.array(mine, dtype=np.int64))

        blk_slot = np.zeros(cfg.nbt_glob, dtype=np.int64)
        for c in range(C):
            for s, b in enumerate(self.core_blocks[c]):
                if b >= 0:
                    blk_slot[b] = s

        counts = np.zeros((C, cfg.nbt, 2), dtype=np.int64)
        percore = []
        for c in range(C):
            m = blk_core[gblock] == c
            r, t = row[m], col[m]
            slot = blk_slot[gblock[m]]
            half = (r >= cfg.split).astype(np.int64)
            o = np.lexsort((r, half, slot))
            r, t, slot, half = r[o], t[o], slot[o], half[o]
            gid = slot * 2 + half
            cnt = np.bincount(gid, minlength=cfg.nbt * 2).reshape(cfg.nbt, 2)
            counts[c] = cnt
            percore.append((r, t, slot, half, gid, cnt))

        # unified tile counts per (slot, source half) across cores; >=1 so
        # every psum block is written
        self.K = np.maximum(
            np.ceil(counts.max(axis=0) / P).astype(np.int64), 1)  # [nbt,2]
        ntl = int(self.K[:, 0].sum())
        nth = int(self.K[:, 1].sum())
        self.ntiles = [ntl, nth]
        self.n_rec = ntl + nth
        self.sbase = np.stack(
            [np.concatenate([[0], np.cumsum(self.K[:, h])])[:-1] for h in (0, 1)],
            axis=1,
        )  # [nbt, 2]

        # per-core gather index streams + tau/val tables
        self.gidx = []    # (lo[128, ntl*8] i16, hi[128, nth*8])
        self.tauval = []  # [128, 2*n_rec] bf16: tau cols then val cols
        for c in range(C):
            r, t, slot, half, gid, cnt = percore[c]
            run_start = np.concatenate([[0], np.cumsum(cnt.reshape(-1))])[:-1]
            j = np.arange(len(r)) - run_start[gid]
            tile_in_run = j // P
            p_slot = j % P
            stream_tile = self.sbase[slot, half] + tile_in_run
            spos = stream_tile * P + p_slot
            gi = []
            for h in (0, 1):
                nt_h = self.ntiles[h]
                arr = np.zeros(nt_h * P, dtype=np.int64)
                used = np.zeros(nt_h * P, dtype=bool)
                m = half == h
                src = r[m] - (cfg.split if h else 0)
                arr[spos[m]] = src
                used[spos[m]] = True
                # trailing pad slots of each dma_gather call -> -1 (the Q7
                # kernel trims trailing negatives: no packets)
                if False:
                    nch = (nt_h + CH - 1) // CH
                    for ci in range(nch):
                        a, b_ = ci * CH * P, min((ci + 1) * CH, nt_h) * P
                        u = np.nonzero(used[a:b_])[0]
                        last = (u[-1] + 1) if len(u) else 0
                        arr[a + last: b_] = -1
                w = arr.astype(np.int16).reshape(-1, 16).T
                gi.append(np.tile(w, (8, 1)).copy())
            self.gidx.append(gi)

            col_of = stream_tile + np.where(half == 1, ntl, 0)
            tau = np.full((P, self.n_rec), -1.0, dtype=np.float32)
            val = np.zeros((P, self.n_rec), dtype=np.float32)
            tau[p_slot, col_of] = (t % TB).astype(np.float32)
            val[p_slot, col_of] = (self.dinv[t] * self.dinv[r]).astype(
                np.float32)
            self.tauval.append(np.ascontiguousarray(
                np.concatenate([tau, val], axis=1).astype(BF16)))


def build_nc(cfg: Cfg, plan: Plan) -> bass.Bass:
    n_rec, TB, DO, DI, CH = plan.n_rec, cfg.tb, cfg.do, cfg.di, cfg.gather_chunk
    ntl = plan.ntiles[0]
    f32, bf16, i16 = mybir.dt.float32, mybir.dt.bfloat16, mybir.dt.int16

    nc = bacc.Bacc("TRN2", target_bir_lowering=False, debug=False,
                   num_swdge_queues=4)
    x_lo = nc.dram_tensor("x_lo", [cfg.nlo, DI], bf16, kind="ExternalInput")
    x_hi = nc.dram_tensor("x_hi", [cfg.nhi_pad, DI], bf16, kind="ExternalInput")
    w = nc.dram_tensor("w", [P, cfg.kc * DO], bf16, kind="ExternalInput")
    bias = nc.dram_tensor("bias", [P, DO], f32, kind="ExternalInput")
    gilo = nc.dram_tensor("gilo", [P, plan.ntiles[0] * 8], i16, kind="ExternalInput")
    gihi = nc.dram_tensor("gihi", [P, plan.ntiles[1] * 8], i16, kind="ExternalInput")
    tauval = nc.dram_tensor("tauval", [P, 2 * n_rec], bf16, kind="ExternalInput")
    ramp = nc.dram_tensor("ramp", [P, TB], bf16, kind="ExternalInput")
    out = nc.dram_tensor("out", [cfg.nbt * P, DO], f32, kind="ExternalOutput")

    nc.gpsimd.load_library(library_config.mlp)

    with tile.TileContext(nc) as tc:
        with (
            tc.tile_pool(name="consts", bufs=1) as consts,
            tc.tile_pool(name="glo", bufs=7) as glo_pool,
            tc.tile_pool(name="ghi", bufs=7) as ghi_pool,
            tc.tile_pool(name="gidx", bufs=4) as gidx_pool,
            tc.tile_pool(name="bsel", bufs=3) as bsel_pool,
            tc.tile_pool(name="zp", bufs=2) as z_pool,
            tc.tile_pool(name="ztp", bufs=2) as zt_pool,
            tc.tile_pool(name="agg_psum", bufs=3, space="PSUM") as aps,
            tc.tile_pool(name="tp_psum", bufs=2, space="PSUM") as tps,
            tc.tile_pool(name="o_psum", bufs=2, space="PSUM") as ops_pool,
            tc.tile_pool(name="ost", bufs=2) as ost_pool,
        ):
            # ---- constants ----
            w_sb = consts.tile([P, cfg.kc * DO], bf16, tag="w")
            nc.sync.dma_start(w_sb[:], w[:, :])
            bias_sb = consts.tile([P, DO], f32, tag="bias")
            nc.sync.dma_start(bias_sb[:], bias[:, :])
            tv_sb = consts.tile([P, 2 * n_rec], bf16, tag="tauval")
            nc.sync.dma_start(tv_sb[:], tauval[:, :])
            ramp_sb = consts.tile([P, TB], bf16, tag="ramp")
            nc.sync.dma_start(ramp_sb[:], ramp[:, :])
            ident = consts.tile([P, P], bf16, tag="ident")
            masks.make_identity(nc, ident[:])

            out_r = out[:, :].rearrange("(a p) f -> p a f", p=P)
            gsrc = [x_lo, x_hi]
            gidx_dram = [gilo, gihi]
            gpools = [glo_pool, ghi_pool]
            nchunks = [(plan.ntiles[h] + CH - 1) // CH for h in (0, 1)]
            gtiles = [[None] * nchunks[0], [None] * nchunks[1]]
            gq = [0]  # rotating SWDGE queue
            IB = 16   # gather chunks of idx per idx-DMA
            gidx_tiles = [{}, {}]

            def ensure_gidx(h, bi):
                if bi in gidx_tiles[h]:
                    return
                c0 = bi * IB * CH
                cw = min(IB * CH, plan.ntiles[h] - c0)
                gi_sb = gidx_pool.tile([P, IB * CH * 8], i16, tag="gi",
                                       name="gi_sb")
                nc.sync.dma_start(gi_sb[:, : cw * 8],
                                  gidx_dram[h][:, c0 * 8: (c0 + cw) * 8])
                gidx_tiles[h][bi] = gi_sb

            def ensure_gchunk(h, ci):
                if gtiles[h][ci] is not None:
                    return
                c0 = ci * CH
                cw = min(CH, plan.ntiles[h] - c0)
                ensure_gidx(h, ci // IB)
                gi_sb = gidx_tiles[h][ci // IB][
                    :, (ci % IB) * CH * 8: (ci % IB) * CH * 8 + cw * 8]
                g_sb = gpools[h].tile([P, CH * DI], bf16, tag=f"g{h}",
                                      name="g_sb")
                nc.gpsimd.dma_gather(
                    g_sb[:, : cw * DI].rearrange("p (t f) -> p t f", f=DI),
                    gsrc[h][:, :],
                    gi_sb,
                    cw * P,
                    cw * P,
                    DI,
                    single_packet=True,
                    queue_num=gq[0],
                )
                gq[0] = (gq[0] + 1) % 4
                gtiles[h][ci] = g_sb

            # on-device B tiles in bulk chunks:
            # B[p, t, tau] = (ramp[tau] == tau[p, t]) * val[p, t]
            BCH = 32
            nbchunks = (n_rec + BCH - 1) // BCH
            btiles = [None] * nbchunks

            def ensure_bchunk(bi):
                if btiles[bi] is not None:
                    return
                c0 = bi * BCH
                cw = min(BCH, n_rec - c0)
                b_sb = bsel_pool.tile([P, BCH * TB], bf16, tag="bsel",
                                      name="b_sb")
                view = b_sb[:, : cw * TB].rearrange("p (t f) -> p t f", f=TB)
                nc.vector.tensor_tensor(
                    view,
                    ramp_sb[:, None, :].to_broadcast([P, cw, TB]),
                    tv_sb[:, c0: c0 + cw, None].to_broadcast([P, cw, TB]),
                    mybir.AluOpType.is_equal,
                )
                nc.vector.tensor_tensor(
                    view, view,
                    tv_sb[:, n_rec + c0: n_rec + c0 + cw, None].to_broadcast(
                        [P, cw, TB]),
                    mybir.AluOpType.mult,
                )
                btiles[bi] = b_sb

            def emit_agg(s):
                """All message-tile matmuls of block s into one [P, DI] psum."""
                psum_s = aps.tile([P, DI], f32, tag="ap", name="apsum")
                kl, kh = plan.K[s, 0], plan.K[s, 1]
                for h, kk in ((0, kl), (1, kh)):
                    for k in range(kk):
                        st = int(plan.sbase[s, h]) + k
                        gt = st + (ntl if h else 0)
                        ensure_gchunk(h, st // CH)
                        ensure_bchunk(gt // BCH)
                        m_ap = gtiles[h][st // CH][
                            :, (st % CH) * DI: (st % CH + 1) * DI]
                        b_ap = btiles[gt // BCH][
                            :, (gt % BCH) * TB: (gt % BCH + 1) * TB]
                        nc.tensor.matmul(
                            psum_s[:, :], b_ap, m_ap,
                            start=(h == 0 and k == 0),
                            stop=(h == 1 and k == kh - 1),
                        )
                z = z_pool.tile([P, DI], bf16, tag="z", name="z_sb")
                nc.vector.tensor_copy(z[:, :], psum_s[:, :])
                return z

            ost = [None]
            base = [0]

            def emit_tail(s, z):
                """Z -> transpose -> @W -> +bias -> relu -> out batch."""
                zt = zt_pool.tile([P, DI], bf16, tag="zt", name="zt_sb")
                for k in range(cfg.kc):
                    tp = tps.tile([P, P], bf16, tag="tp", name="tpsum")
                    nc.tensor.transpose(
                        tp[:, :], z[:, k * P: (k + 1) * P], ident[:, :])
                    nc.scalar.copy(zt[:, k * P: (k + 1) * P], tp[:, :])
                opsum = ops_pool.tile([P, DO], f32, tag="op", name="opsum")
                for k in range(cfg.kc):
                    nc.tensor.matmul(
                        opsum[:, :],
                        zt[:, k * P: (k + 1) * P],
                        w_sb[:, k * DO: (k + 1) * DO],
                        start=(k == 0), stop=(k == cfg.kc - 1),
                    )
                if ost[0] is None:
                    ost[0] = ost_pool.tile([P, cfg.out_batch * DO], f32,
                                           tag="ost", name="ost")
                    base[0] = s
                osl = ost[0][:, (s - base[0]) * DO: (s - base[0] + 1) * DO]
                nc.vector.tensor_add(osl, opsum[:, :], bias_sb[:, :])
                nc.scalar.activation(osl, osl,
                                     mybir.ActivationFunctionType.Relu)
                if s - base[0] + 1 == cfg.out_batch or s == cfg.nbt - 1:
                    cnt = s - base[0] + 1
                    # NOTE: HWDGE writes to the ExternalOutput while SWDGE
                    # gathers are in flight crash the device (NRT 101);
                    # route output writes through SWDGE (gpsimd).
                    nc.gpsimd.dma_start(
                        out_r[:, base[0]: base[0] + cnt, :],
                        ost[0][:, : cnt * DO].rearrange(
                            "p (a f) -> p a f", f=DO),
                    )
                    ost[0] = None

            # software pipeline: agg(s+1) runs on PE before tail(s)
            prev = None
            for s in range(cfg.nbt):
                z = emit_agg(s)
                if prev is not None:
                    emit_tail(prev[0], prev[1])
                prev = (s, z)
            emit_tail(prev[0], prev[1])
    nc.compile()
    return nc


def _prep_shared(cfg: Cfg, x, W, b):
    xpad = np.zeros((cfg.npad, cfg.di), dtype=BF16)
    xpad[: cfg.n] = x.astype(BF16)
    x_lo = np.ascontiguousarray(xpad[: cfg.nlo])
    x_hi = np.ascontiguousarray(xpad[cfg.split:])
    w_host = np.ascontiguousarray(
        W.astype(BF16).reshape(cfg.kc, P, cfg.do).transpose(1, 0, 2).reshape(P, -1)
    )
    bias = np.ascontiguousarray(
        np.broadcast_to(b.astype(np.float32), (P, cfg.do)))
    ramp = np.ascontiguousarray(
        np.broadcast_to(np.arange(cfg.tb, dtype=np.float32), (P, cfg.tb))
    ).astype(BF16)
    return x_lo, x_hi, w_host, bias, ramp


def run(cfg: Cfg, x, edge_index, W, b, trace=False):
    plan = Plan(cfg, edge_index)
    nc = build_nc(cfg, plan)
    x_lo, x_hi, w_host, bias, ramp = _prep_shared(cfg, x, W, b)
    in_maps = []
    for c in range(cfg.cores):
        in_maps.append({
            "x_lo": x_lo, "x_hi": x_hi, "w": w_host, "bias": bias,
            "ramp": ramp,
            "gilo": plan.gidx[c][0], "gihi": plan.gidx[c][1],
            "tauval": plan.tauval[c],
        })
    res = run_bass_kernel_spmd(nc, in_maps, core_ids=list(range(cfg.cores)),
                               trace=trace)
    out = np.zeros((cfg.n, cfg.do), dtype=np.float32)
    for c in range(cfg.cores):
        o = res.results[c]["out"]
        for s, gb in enumerate(plan.core_blocks[c]):
            if gb < 0:
                continue
            t0 = gb * cfg.tb
            t1 = min(t0 + cfg.tb, cfg.n)
            out[t0:t1] = o[s * cfg.tb: s * cfg.tb + (t1 - t0)]
    return out, res


FULL = Cfg(n=50000, e=800000, di=512, do=256, cores=8)


def kernel(x, edge_index, W, b):
    out, _ = run(FULL, np.asarray(x), np.asarray(edge_index), np.asarray(W),
                 np.asarray(b))
    return out


# revision 28
# speedup vs baseline: 1.2512x; 1.2512x over previous
"""GCN layer (gnn_message_passing) on 8 Trainium2 NeuronCores.

out = relu(D^-1/2 (A+I) D^-1/2 (x @ W) + b)

Key identity: sum_msgs norm * (xW)[src] = (sum_msgs norm * x[src]) @ W.
Aggregation runs in x-space, so NO full-graph GEMM exists at all: each core
gathers raw x rows for its messages, scatter-accumulates them per 128-target
block on the PE (selection-matrix matmul), and applies W only to its own
aggregated targets (1/8 of the nodes).  Per core:

  - Target blocks of 128 nodes are load-balanced across cores (LPT on
    message count); each core owns 49 blocks (6272 padded target rows).
  - dma_gather pulls x rows (1KB, bf16) for message tiles of 128 straight
    from the x input halves (int16 index range); round-robin over all 4
    SWDGE queues keeps the 4 Q7 descriptor-gen pairs concurrent.  No
    producer dependency: gathers start at t=0.
  - B tiles (B[slot, tau] = dinv[t]*dinv[s] one-hot over the block's 128
    targets) are built on device in bulk: (ramp == tau) * val with
    broadcast tensor_tensor ops from tiny bf16 tau/val tables.
  - Per block: PSUM [128 targets, 512] accumulates B^T @ X_msgs over all
    the block's tiles (lo+hi source halves back-to-back), then
    Z -> PE-transpose -> Z^T @ W -> +bias -> ReLU -> out, software
    pipelined one block deep so the PE never waits on the fixups.

All planning is computed on the host from the integer edge list only; all
floating-point work runs on device.
"""

import ml_dtypes
import numpy as np

import concourse.bacc as bacc
import concourse.bass as bass
import concourse.mybir as mybir
import concourse.tile as tile
from concourse import library_config, masks
from concourse.bass_utils import run_bass_kernel_spmd

BF16 = ml_dtypes.bfloat16
P = 128  # partitions


class Cfg:
    def __init__(self, n, e, di, do, cores, gather_chunk=8, out_batch=8):
        self.n, self.e, self.di, self.do, self.cores = n, e, di, do, cores
        self.tb = P                       # targets per psum block
        self.gather_chunk = gather_chunk  # message tiles per dma_gather call
        self.out_batch = out_batch        # blocks per output DMA
        self.nbt_glob = (n + P - 1) // P          # global target blocks
        self.nbt = (self.nbt_glob + cores - 1) // cores  # blocks per core
        self.nt = self.nbt * P            # padded targets per core
        self.nblocks = (n + P - 1) // P   # node blocks (for x padding)
        self.npad = self.nblocks * P
        self.split_blocks = (self.nblocks + 1) // 2
        self.split = self.split_blocks * P          # lo/hi node boundary
        self.nlo = self.split
        self.nhi_pad = self.npad - self.split
        assert self.nlo <= 32768 and self.nhi_pad <= 32768
        assert di % P == 0
        self.kc = di // P                 # contraction chunks for W


class Plan:
    """Host-side integer/index preprocessing: shared structure (uniform
    across cores, baked into the program) + per-core index/tau/val data."""

    def __init__(self, cfg: Cfg, edge_index: np.ndarray):
        n, C, TB, CH = cfg.n, cfg.cores, cfg.tb, cfg.gather_chunk
        row = np.asarray(edge_index[0], dtype=np.int64)
        col = np.asarray(edge_index[1], dtype=np.int64)
        loop = np.arange(n, dtype=np.int64)
        row = np.concatenate([row, loop])
        col = np.concatenate([col, loop])
        deg = np.bincount(col, minlength=n).astype(np.float64)
        self.dinv = (1.0 / np.sqrt(deg)).astype(np.float32)

        # ---- load-balanced assignment of global target blocks to cores ----
        gblock = col // TB
        gcount = np.bincount(gblock, minlength=cfg.nbt_glob)
        order = np.argsort(-gcount, kind="stable")          # LPT order
        core_load = np.zeros(C, dtype=np.int64)
        core_nblk = np.zeros(C, dtype=np.int64)
        blk_core = np.zeros(cfg.nbt_glob, dtype=np.int64)
        for b in order:
            c = np.argmin(np.where(core_nblk < cfg.nbt, core_load, 1 << 60))
            blk_core[b] = c
            core_load[c] += gcount[b]
            core_nblk[c] += 1
        self.core_blocks = []   # core -> [global block id per slot]
        for c in range(C):
            mine = [b for b in order if blk_core[b] == c]
            mine += [-1] * (cfg.nbt - len(mine))            # pad slots
            self.core_blocks.append(np.array(mine, dtype=np.int64))

        blk_slot = np.zeros(cfg.nbt_glob, dtype=np.int64)
        for c in range(C):
            for s, b in enumerate(self.core_blocks[c]):
                if b >= 0:
                    blk_slot[b] = s

        counts = np.zeros((C, cfg.nbt, 2), dtype=np.int64)
        percore = []
        for c in range(C):
            m = blk_core[gblock] == c
            r, t = row[m], col[m]
            slot = blk_slot[gblock[m]]
            half = (r >= cfg.split).astype(np.int64)
            o = np.lexsort((r, half, slot))
            r, t, slot, half = r[o], t[o], slot[o], half[o]
            gid = slot * 2 + half
            cnt = np.bincount(gid, minlength=cfg.nbt * 2).reshape(cfg.nbt, 2)
            counts[c] = cnt
            percore.append((r, t, slot, half, gid, cnt))

        # unified tile counts per (slot, source half) across cores; >=1 so
        # every psum block is written
        self.K = np.maximum(
            np.ceil(counts.max(axis=0) / P).astype(np.int64), 1)  # [nbt,2]
        ntl = int(self.K[:, 0].sum())
        nth = int(self.K[:, 1].sum())
        self.ntiles = [ntl, nth]
        self.n_rec = ntl + nth
        self.sbase = np.stack(
            [np.concatenate([[0], np.cumsum(self.K[:, h])])[:-1] for h in (0, 1)],
            axis=1,
        )  # [nbt, 2]

        # per-core gather index streams + tau/val tables
        self.gidx = []    # (lo[128, ntl*8] i16, hi[128, nth*8])
        self.tauval = []  # [128, 2*n_rec] bf16: tau cols then val cols
        for c in range(C):
            r, t, slot, half, gid, cnt = percore[c]
            run_start = np.concatenate([[0], np.cumsum(cnt.reshape(-1))])[:-1]
            j = np.arange(len(r)) - run_start[gid]
            tile_in_run = j // P
            p_slot = j % P
            stream_tile = self.sbase[slot, half] + tile_in_run
            spos = stream_tile * P + p_slot
            gi = []
            for h in (0, 1):
                nt_h = self.ntiles[h]
                arr = np.zeros(nt_h * P, dtype=np.int64)
                used = np.zeros(nt_h * P, dtype=bool)
                m = half == h
                src = r[m] - (cfg.split if h else 0)
                arr[spos[m]] = src
                used[spos[m]] = True
                # trailing pad slots of each dma_gather call -> -1 (the Q7
                # kernel trims trailing negatives: no packets)
                if False:
                    nch = (nt_h + CH - 1) // CH
                    for ci in range(nch):
                        a, b_ = ci * CH * P, min((ci + 1) * CH, nt_h) * P
                        u = np.nonzero(used[a:b_])[0]
                        last = (u[-1] + 1) if len(u) else 0
                        arr[a + last: b_] = -1
                w = arr.astype(np.int16).reshape(-1, 16).T
                gi.append(np.tile(w, (8, 1)).copy())
            self.gidx.append(gi)

            col_of = stream_tile + np.where(half == 1, ntl, 0)
            tau = np.full((P, self.n_rec), -1.0, dtype=np.float32)
            val = np.zeros((P, self.n_rec), dtype=np.float32)
            tau[p_slot, col_of] = (t % TB).astype(np.float32)
            val[p_slot, col_of] = (self.dinv[t] * self.dinv[r]).astype(
                np.float32)
            self.tauval.append(np.ascontiguousarray(
                np.concatenate([tau, val], axis=1).astype(BF16)))


def build_nc(cfg: Cfg, plan: Plan) -> bass.Bass:
    n_rec, TB, DO, DI, CH = plan.n_rec, cfg.tb, cfg.do, cfg.di, cfg.gather_chunk
    ntl = plan.ntiles[0]
    f32, bf16, i16 = mybir.dt.float32, mybir.dt.bfloat16, mybir.dt.int16

    nc = bacc.Bacc("TRN2", target_bir_lowering=False, debug=False,
                   num_swdge_queues=4)
    x_lo = nc.dram_tensor("x_lo", [cfg.nlo, DI], bf16, kind="ExternalInput")
    x_hi = nc.dram_tensor("x_hi", [cfg.nhi_pad, DI], bf16, kind="ExternalInput")
    w = nc.dram_tensor("w", [P, cfg.kc * DO], bf16, kind="ExternalInput")
    bias = nc.dram_tensor("bias", [P, DO], f32, kind="ExternalInput")
    gilo = nc.dram_tensor("gilo", [P, plan.ntiles[0] * 8], i16, kind="ExternalInput")
    gihi = nc.dram_tensor("gihi", [P, plan.ntiles[1] * 8], i16, kind="ExternalInput")
    tauval = nc.dram_tensor("tauval", [P, 2 * n_rec], bf16, kind="ExternalInput")
    ramp = nc.dram_tensor("ramp", [P, TB], bf16, kind="ExternalInput")
    out = nc.dram_tensor("out", [cfg.nbt * P, DO], f32, kind="ExternalOutput")

    nc.gpsimd.load_library(library_config.mlp)

    with tile.TileContext(nc) as tc:
        with (
            tc.tile_pool(name="consts", bufs=1) as consts,
            tc.tile_pool(name="glo", bufs=7) as glo_pool,
            tc.tile_pool(name="ghi", bufs=7) as ghi_pool,
            tc.tile_pool(name="gidx", bufs=4) as gidx_pool,
            tc.tile_pool(name="bsel", bufs=3) as bsel_pool,
            tc.tile_pool(name="zp", bufs=2) as z_pool,
            tc.tile_pool(name="ztp", bufs=2) as zt_pool,
            tc.tile_pool(name="agg_psum", bufs=3, space="PSUM") as aps,
            tc.tile_pool(name="tp_psum", bufs=2, space="PSUM") as tps,
            tc.tile_pool(name="o_psum", bufs=2, space="PSUM") as ops_pool,
            tc.tile_pool(name="ost", bufs=2) as ost_pool,
        ):
            # ---- constants ----
            w_sb = consts.tile([P, cfg.kc * DO], bf16, tag="w")
            nc.sync.dma_start(w_sb[:], w[:, :])
            bias_sb = consts.tile([P, DO], f32, tag="bias")
            nc.sync.dma_start(bias_sb[:], bias[:, :])
            tv_sb = consts.tile([P, 2 * n_rec], bf16, tag="tauval")
            nc.sync.dma_start(tv_sb[:], tauval[:, :])
            ramp_sb = consts.tile([P, TB], bf16, tag="ramp")
            nc.sync.dma_start(ramp_sb[:], ramp[:, :])
            ident = consts.tile([P, P], bf16, tag="ident")
            masks.make_identity(nc, ident[:])

            out_r = out[:, :].rearrange("(a p) f -> p a f", p=P)
            gsrc = [x_lo, x_hi]
            gidx_dram = [gilo, gihi]
            gpools = [glo_pool, ghi_pool]
            nchunks = [(plan.ntiles[h] + CH - 1) // CH for h in (0, 1)]
            gtiles = [[None] * nchunks[0], [None] * nchunks[1]]
            gq = [0]  # rotating SWDGE queue
            IB = 16   # gather chunks of idx per idx-DMA
            gidx_tiles = [{}, {}]

            def ensure_gidx(h, bi):
                if bi in gidx_tiles[h]:
                    return
                c0 = bi * IB * CH
                cw = min(IB * CH, plan.ntiles[h] - c0)
                gi_sb = gidx_pool.tile([P, IB * CH * 8], i16, tag="gi",
                                       name="gi_sb")
                nc.sync.dma_start(gi_sb[:, : cw * 8],
                                  gidx_dram[h][:, c0 * 8: (c0 + cw) * 8])
                gidx_tiles[h][bi] = gi_sb

            def ensure_gchunk(h, ci):
                if gtiles[h][ci] is not None:
                    return
                c0 = ci * CH
                cw = min(CH, plan.ntiles[h] - c0)
                ensure_gidx(h, ci // IB)
                gi_sb = gidx_tiles[h][ci // IB][
                    :, (ci % IB) * CH * 8: (ci % IB) * CH * 8 + cw * 8]
                g_sb = gpools[h].tile([P, CH * DI], bf16, tag=f"g{h}",
                                      name="g_sb")
                nc.gpsimd.dma_gather(
                    g_sb[:, : cw * DI].rearrange("p (t f) -> p t f", f=DI),
                    gsrc[h][:, :],
                    gi_sb,
                    cw * P,
                    cw * P,
                    DI,
                    single_packet=True,
                    queue_num=gq[0],
                )
                gq[0] = (gq[0] + 1) % 4
                gtiles[h][ci] = g_sb

            # on-device B tiles in bulk chunks:
            # B[p, t, tau] = (ramp[tau] == tau[p, t]) * val[p, t]
            BCH = 32
            nbchunks = (n_rec + BCH - 1) // BCH
            btiles = [None] * nbchunks

            def ensure_bchunk(bi):
                if btiles[bi] is not None:
                    return
                c0 = bi * BCH
                cw = min(BCH, n_rec - c0)
                b_sb = bsel_pool.tile([P, BCH * TB], bf16, tag="bsel",
                                      name="b_sb")
                view = b_sb[:, : cw * TB].rearrange("p (t f) -> p t f", f=TB)
                nc.vector.tensor_tensor(
                    view,
                    ramp_sb[:, None, :].to_broadcast([P, cw, TB]),
                    tv_sb[:, c0: c0 + cw, None].to_broadcast([P, cw, TB]),
                    mybir.AluOpType.is_equal,
                )
                nc.vector.tensor_tensor(
                    view, view,
                    tv_sb[:, n_rec + c0: n_rec + c0 + cw, None].to_broadcast(
                        [P, cw, TB]),
                    mybir.AluOpType.mult,
                )
                btiles[bi] = b_sb

            def emit_agg(s):
                """All message-tile matmuls of block s into one [P, DI] psum."""
                psum_s = aps.tile([P, DI], f32, tag="ap", name="apsum")
                kl, kh = plan.K[s, 0], plan.K[s, 1]
                for h, kk in ((0, kl), (1, kh)):
                    for k in range(kk):
                        st = int(plan.sbase[s, h]) + k
                        gt = st + (ntl if h else 0)
                        ensure_gchunk(h, st // CH)
                        ensure_bchunk(gt // BCH)
                        m_ap = gtiles[h][st // CH][
                            :, (st % CH) * DI: (st % CH + 1) * DI]
                        b_ap = btiles[gt // BCH][
                            :, (gt % BCH) * TB: (gt % BCH + 1) * TB]
                        nc.tensor.matmul(
                            psum_s[:, :], b_ap, m_ap,
                            start=(h == 0 and k == 0),
                            stop=(h == 1 and k == kh - 1),
                        )
                z = z_pool.tile([P, DI], bf16, tag="z", name="z_sb")
                nc.vector.tensor_copy(z[:, :], psum_s[:, :])
                return z

            ost = [None]
            base = [0]

            def emit_tail(s, z):
                """Z -> transpose -> @W -> +bias -> relu -> out batch."""
                zt = zt_pool.tile([P, DI], bf16, tag="zt", name="zt_sb")
                for k in range(cfg.kc):
                    tp = tps.tile([P, P], bf16, tag="tp", name="tpsum")
                    nc.tensor.transpose(
                        tp[:, :], z[:, k * P: (k + 1) * P], ident[:, :])
                    nc.scalar.copy(zt[:, k * P: (k + 1) * P], tp[:, :])
                opsum = ops_pool.tile([P, DO], f32, tag="op", name="opsum")
                for k in range(cfg.kc):
                    nc.tensor.matmul(
                        opsum[:, :],
                        zt[:, k * P: (k + 1) * P],
                        w_sb[:, k * DO: (k + 1) * DO],
                        start=(k == 0), stop=(k == cfg.kc - 1),
                    )
                if ost[0] is None:
                    ost[0] = ost_pool.tile([P, cfg.out_batch * DO], f32,
                                           tag="ost", name="ost")
                    base[0] = s
                osl = ost[0][:, (s - base[0]) * DO: (s - base[0] + 1) * DO]
                nc.vector.tensor_add(osl, opsum[:, :], bias_sb[:, :])
                nc.scalar.activation(osl, osl,
                                     mybir.ActivationFunctionType.Relu)
                if s - base[0] + 1 == cfg.out_batch or s == cfg.nbt - 1:
                    cnt = s - base[0] + 1
                    # NOTE: HWDGE writes to the ExternalOutput while SWDGE
                    # gathers are in flight crash the device (NRT 101);
                    # route output writes through SWDGE (gpsimd).
                    nc.gpsimd.dma_start(
                        out_r[:, base[0]: base[0] + cnt, :],
                        ost[0][:, : cnt * DO].rearrange(
                            "p (a f) -> p a f", f=DO),
                    )
                    ost[0] = None

            # software pipeline: agg(s+1) runs on PE before tail(s)
            prev = None
            for s in range(cfg.nbt):
                z = emit_agg(s)
                if prev is not None:
                    emit_tail(prev[0], prev[1])
                prev = (s, z)
            emit_tail(prev[0], prev[1])
    nc.compile()
    return nc


def _prep_shared(cfg: Cfg, x, W, b):
    xpad = np.zeros((cfg.npad, cfg.di), dtype=BF16)
    xpad[: cfg.n] = x.astype(BF16)
    x_lo = np.ascontiguousarray(xpad[: cfg.nlo])
    x_hi = np.ascontiguousarray(xpad[cfg.split:])
    w_host = np.ascontiguousarray(
        W.astype(BF16).reshape(cfg.kc, P, cfg.do).transpose(1, 0, 2).reshape(P, -1)
    )
    bias = np.ascontiguousarray(
        np.broadcast_to(b.astype(np.float32), (P, cfg.do)))
    ramp = np.ascontiguousarray(
        np.broadcast_to(np.arange(cfg.tb, dtype=np.float32), (P, cfg.tb))
    ).astype(BF16)
    return x_lo, x_hi, w_host, bias, ramp


def run(cfg: Cfg, x, edge_index, W, b, trace=False):
    plan = Plan(cfg, edge_index)
    nc = build_nc(cfg, plan)
    x_lo, x_hi, w_host, bias, ramp = _prep_shared(cfg, x, W, b)
    in_maps = []
    for c in range(cfg.cores):
        in_maps.append({
            "x_lo": x_lo, "x_hi": x_hi, "w": w_host, "bias": bias,
            "ramp": ramp,
            "gilo": plan.gidx[c][0], "gihi": plan.gidx[c][1],
            "tauval": plan.tauval[c],
        })
    res = run_bass_kernel_spmd(nc, in_maps, core_ids=list(range(cfg.cores)),
                               trace=trace)
    out = np.zeros((cfg.n, cfg.do), dtype=np.float32)
    for c in range(cfg.cores):
        o = res.results[c]["out"]
        for s, gb in enumerate(plan.core_blocks[c]):
            if gb < 0:
                continue
            t0 = gb * cfg.tb
            t1 = min(t0 + cfg.tb, cfg.n)
            out[t0:t1] = o[s * cfg.tb: s * cfg.tb + (t1 - t0)]
    return out, res


FULL = Cfg(n=50000, e=800000, di=512, do=256, cores=8)


def kernel(x, edge_index, W, b):
    out, _ = run(FULL, np.asarray(x), np.asarray(edge_index), np.asarray(W),
                 np.asarray(b))
    return out


# revision 29
# speedup vs baseline: 1.3417x; 1.0723x over previous
"""GCN layer (gnn_message_passing) on 8 Trainium2 NeuronCores.

out = relu(D^-1/2 (A+I) D^-1/2 (x @ W) + b)

Key identity: sum_msgs norm * (xW)[src] = (sum_msgs norm * x[src]) @ W.
Aggregation runs in x-space, so NO full-graph GEMM exists at all: each core
gathers raw x rows for its messages, scatter-accumulates them per 128-target
block on the PE (selection-matrix matmul), and applies W only to its own
aggregated targets (1/8 of the nodes).  Per core:

  - Target blocks of 128 nodes are load-balanced across cores (LPT on
    message count); each core owns 49 blocks (6272 padded target rows).
  - dma_gather pulls x rows (1KB, bf16) for message tiles of 128 straight
    from the x input halves (int16 index range); round-robin over all 4
    SWDGE queues keeps the 4 Q7 descriptor-gen pairs concurrent.  No
    producer dependency: gathers start at t=0.
  - B tiles (B[slot, tau] = dinv[t]*dinv[s] one-hot over the block's 128
    targets) are built on device in bulk: (ramp == tau) * val with
    broadcast tensor_tensor ops from tiny bf16 tau/val tables.
  - Per block: PSUM [128 targets, 512] accumulates B^T @ X_msgs over all
    the block's tiles (lo+hi source halves back-to-back), then
    Z -> PE-transpose -> Z^T @ W -> +bias -> ReLU -> out, software
    pipelined one block deep so the PE never waits on the fixups.

All planning is computed on the host from the integer edge list only; all
floating-point work runs on device.
"""

import ml_dtypes
import numpy as np

import concourse.bacc as bacc
import concourse.bass as bass
import concourse.mybir as mybir
import concourse.tile as tile
from concourse import library_config, masks
from concourse.bass_utils import run_bass_kernel_spmd

BF16 = ml_dtypes.bfloat16
P = 128  # partitions


class Cfg:
    def __init__(self, n, e, di, do, cores, gather_chunk=8, out_batch=12):
        self.n, self.e, self.di, self.do, self.cores = n, e, di, do, cores
        self.tb = P                       # targets per psum block
        self.gather_chunk = gather_chunk  # message tiles per dma_gather call
        self.out_batch = out_batch        # blocks per output DMA
        self.nbt_glob = (n + P - 1) // P          # global target blocks
        self.nbt = (self.nbt_glob + cores - 1) // cores  # blocks per core
        self.nt = self.nbt * P            # padded targets per core
        self.nblocks = (n + P - 1) // P   # node blocks (for x padding)
        self.npad = self.nblocks * P
        self.split_blocks = (self.nblocks + 1) // 2
        self.split = self.split_blocks * P          # lo/hi node boundary
        self.nlo = self.split
        self.nhi_pad = self.npad - self.split
        assert self.nlo <= 32768 and self.nhi_pad <= 32768
        assert di % P == 0
        self.kc = di // P                 # contraction chunks for W


class Plan:
    """Host-side integer/index preprocessing: shared structure (uniform
    across cores, baked into the program) + per-core index/tau/val data."""

    def __init__(self, cfg: Cfg, edge_index: np.ndarray):
        n, C, TB, CH = cfg.n, cfg.cores, cfg.tb, cfg.gather_chunk
        row = np.asarray(edge_index[0], dtype=np.int64)
        col = np.asarray(edge_index[1], dtype=np.int64)
        loop = np.arange(n, dtype=np.int64)
        row = np.concatenate([row, loop])
        col = np.concatenate([col, loop])
        deg = np.bincount(col, minlength=n).astype(np.float64)
        self.dinv = (1.0 / np.sqrt(deg)).astype(np.float32)

        # ---- load-balanced assignment of global target blocks to cores ----
        gblock = col // TB
        gcount = np.bincount(gblock, minlength=cfg.nbt_glob)
        order = np.argsort(-gcount, kind="stable")          # LPT order
        core_load = np.zeros(C, dtype=np.int64)
        core_nblk = np.zeros(C, dtype=np.int64)
        blk_core = np.zeros(cfg.nbt_glob, dtype=np.int64)
        for b in order:
            c = np.argmin(np.where(core_nblk < cfg.nbt, core_load, 1 << 60))
            blk_core[b] = c
            core_load[c] += gcount[b]
            core_nblk[c] += 1
        self.core_blocks = []   # core -> [global block id per slot]
        for c in range(C):
            mine = [b for b in order if blk_core[b] == c]
            mine += [-1] * (cfg.nbt - len(mine))            # pad slots
            self.core_blocks.append(np.array(mine, dtype=np.int64))

        blk_slot = np.zeros(cfg.nbt_glob, dtype=np.int64)
        for c in range(C):
            for s, b in enumerate(self.core_blocks[c]):
                if b >= 0:
                    blk_slot[b] = s

        counts = np.zeros((C, cfg.nbt, 2), dtype=np.int64)
        percore = []
        for c in range(C):
            m = blk_core[gblock] == c
            r, t = row[m], col[m]
            slot = blk_slot[gblock[m]]
            half = (r >= cfg.split).astype(np.int64)
            o = np.lexsort((r, half, slot))
            r, t, slot, half = r[o], t[o], slot[o], half[o]
            gid = slot * 2 + half
            cnt = np.bincount(gid, minlength=cfg.nbt * 2).reshape(cfg.nbt, 2)
            counts[c] = cnt
            percore.append((r, t, slot, half, gid, cnt))

        # unified tile counts per (slot, source half) across cores; >=1 so
        # every psum block is written
        self.K = np.maximum(
            np.ceil(counts.max(axis=0) / P).astype(np.int64), 1)  # [nbt,2]
        ntl = int(self.K[:, 0].sum())
        nth = int(self.K[:, 1].sum())
        self.ntiles = [ntl, nth]
        self.n_rec = ntl + nth
        self.sbase = np.stack(
            [np.concatenate([[0], np.cumsum(self.K[:, h])])[:-1] for h in (0, 1)],
            axis=1,
        )  # [nbt, 2]

        # per-core gather index streams + tau/val tables
        self.gidx = []    # (lo[128, ntl*8] i16, hi[128, nth*8])
        self.tauval = []  # [128, 2*n_rec] bf16: tau cols then val cols
        for c in range(C):
            r, t, slot, half, gid, cnt = percore[c]
            run_start = np.concatenate([[0], np.cumsum(cnt.reshape(-1))])[:-1]
            j = np.arange(len(r)) - run_start[gid]
            tile_in_run = j // P
            p_slot = j % P
            stream_tile = self.sbase[slot, half] + tile_in_run
            spos = stream_tile * P + p_slot
            gi = []
            for h in (0, 1):
                nt_h = self.ntiles[h]
                arr = np.zeros(nt_h * P, dtype=np.int64)
                used = np.zeros(nt_h * P, dtype=bool)
                m = half == h
                src = r[m] - (cfg.split if h else 0)
                arr[spos[m]] = src
                used[spos[m]] = True
                # trailing pad slots of each dma_gather call -> -1 (the Q7
                # kernel trims trailing negatives: no packets)
                if False:
                    nch = (nt_h + CH - 1) // CH
                    for ci in range(nch):
                        a, b_ = ci * CH * P, min((ci + 1) * CH, nt_h) * P
                        u = np.nonzero(used[a:b_])[0]
                        last = (u[-1] + 1) if len(u) else 0
                        arr[a + last: b_] = -1
                w = arr.astype(np.int16).reshape(-1, 16).T
                gi.append(np.tile(w, (8, 1)).copy())
            self.gidx.append(gi)

            col_of = stream_tile + np.where(half == 1, ntl, 0)
            tau = np.full((P, self.n_rec), -1.0, dtype=np.float32)
            val = np.zeros((P, self.n_rec), dtype=np.float32)
            tau[p_slot, col_of] = (t % TB).astype(np.float32)
            val[p_slot, col_of] = (self.dinv[t] * self.dinv[r]).astype(
                np.float32)
            self.tauval.append(np.ascontiguousarray(
                np.concatenate([tau, val], axis=1).astype(BF16)))


def build_nc(cfg: Cfg, plan: Plan) -> bass.Bass:
    n_rec, TB, DO, DI, CH = plan.n_rec, cfg.tb, cfg.do, cfg.di, cfg.gather_chunk
    ntl = plan.ntiles[0]
    f32, bf16, i16 = mybir.dt.float32, mybir.dt.bfloat16, mybir.dt.int16

    nc = bacc.Bacc("TRN2", target_bir_lowering=False, debug=False,
                   num_swdge_queues=4)
    x_lo = nc.dram_tensor("x_lo", [cfg.nlo, DI], bf16, kind="ExternalInput")
    x_hi = nc.dram_tensor("x_hi", [cfg.nhi_pad, DI], bf16, kind="ExternalInput")
    w = nc.dram_tensor("w", [P, cfg.kc * DO], bf16, kind="ExternalInput")
    bias = nc.dram_tensor("bias", [P, DO], f32, kind="ExternalInput")
    gilo = nc.dram_tensor("gilo", [P, plan.ntiles[0] * 8], i16, kind="ExternalInput")
    gihi = nc.dram_tensor("gihi", [P, plan.ntiles[1] * 8], i16, kind="ExternalInput")
    tauval = nc.dram_tensor("tauval", [P, 2 * n_rec], bf16, kind="ExternalInput")
    ramp = nc.dram_tensor("ramp", [P, TB], bf16, kind="ExternalInput")
    out = nc.dram_tensor("out", [cfg.nbt * P, DO], f32, kind="ExternalOutput")

    nc.gpsimd.load_library(library_config.mlp)

    with tile.TileContext(nc) as tc:
        with (
            tc.tile_pool(name="consts", bufs=1) as consts,
            tc.tile_pool(name="glo", bufs=7) as glo_pool,
            tc.tile_pool(name="ghi", bufs=7) as ghi_pool,
            tc.tile_pool(name="gidx", bufs=4) as gidx_pool,
            tc.tile_pool(name="bsel", bufs=3) as bsel_pool,
            tc.tile_pool(name="zp", bufs=2) as z_pool,
            tc.tile_pool(name="ztp", bufs=2) as zt_pool,
            tc.tile_pool(name="agg_psum", bufs=3, space="PSUM") as aps,
            tc.tile_pool(name="tp_psum", bufs=2, space="PSUM") as tps,
            tc.tile_pool(name="o_psum", bufs=2, space="PSUM") as ops_pool,
            tc.tile_pool(name="ost", bufs=2) as ost_pool,
        ):
            # ---- constants ----
            w_sb = consts.tile([P, cfg.kc * DO], bf16, tag="w")
            nc.sync.dma_start(w_sb[:], w[:, :])
            bias_sb = consts.tile([P, DO], f32, tag="bias")
            nc.sync.dma_start(bias_sb[:], bias[:, :])
            tv_sb = consts.tile([P, 2 * n_rec], bf16, tag="tauval")
            nc.sync.dma_start(tv_sb[:], tauval[:, :])
            ramp_sb = consts.tile([P, TB], bf16, tag="ramp")
            nc.sync.dma_start(ramp_sb[:], ramp[:, :])
            ident = consts.tile([P, P], bf16, tag="ident")
            masks.make_identity(nc, ident[:])

            out_r = out[:, :].rearrange("(a p) f -> p a f", p=P)
            gsrc = [x_lo, x_hi]
            gidx_dram = [gilo, gihi]
            gpools = [glo_pool, ghi_pool]
            nchunks = [(plan.ntiles[h] + CH - 1) // CH for h in (0, 1)]
            gtiles = [[None] * nchunks[0], [None] * nchunks[1]]
            gq = [0]  # rotating SWDGE queue
            IB = 16   # gather chunks of idx per idx-DMA
            gidx_tiles = [{}, {}]

            def ensure_gidx(h, bi):
                if bi in gidx_tiles[h]:
                    return
                c0 = bi * IB * CH
                cw = min(IB * CH, plan.ntiles[h] - c0)
                gi_sb = gidx_pool.tile([P, IB * CH * 8], i16, tag="gi",
                                       name="gi_sb")
                nc.sync.dma_start(gi_sb[:, : cw * 8],
                                  gidx_dram[h][:, c0 * 8: (c0 + cw) * 8])
                gidx_tiles[h][bi] = gi_sb

            def ensure_gchunk(h, ci):
                if gtiles[h][ci] is not None:
                    return
                c0 = ci * CH
                cw = min(CH, plan.ntiles[h] - c0)
                ensure_gidx(h, ci // IB)
                gi_sb = gidx_tiles[h][ci // IB][
                    :, (ci % IB) * CH * 8: (ci % IB) * CH * 8 + cw * 8]
                g_sb = gpools[h].tile([P, CH * DI], bf16, tag=f"g{h}",
                                      name="g_sb")
                nc.gpsimd.dma_gather(
                    g_sb[:, : cw * DI].rearrange("p (t f) -> p t f", f=DI),
                    gsrc[h][:, :],
                    gi_sb,
                    cw * P,
                    cw * P,
                    DI,
                    single_packet=True,
                    queue_num=gq[0],
                )
                gq[0] = (gq[0] + 1) % 4
                gtiles[h][ci] = g_sb

            # on-device B tiles in bulk chunks:
            # B[p, t, tau] = (ramp[tau] == tau[p, t]) * val[p, t]
            BCH = 32
            nbchunks = (n_rec + BCH - 1) // BCH
            btiles = [None] * nbchunks

            def ensure_bchunk(bi):
                if btiles[bi] is not None:
                    return
                c0 = bi * BCH
                cw = min(BCH, n_rec - c0)
                b_sb = bsel_pool.tile([P, BCH * TB], bf16, tag="bsel",
                                      name="b_sb")
                view = b_sb[:, : cw * TB].rearrange("p (t f) -> p t f", f=TB)
                nc.vector.tensor_tensor(
                    view,
                    ramp_sb[:, None, :].to_broadcast([P, cw, TB]),
                    tv_sb[:, c0: c0 + cw, None].to_broadcast([P, cw, TB]),
                    mybir.AluOpType.is_equal,
                )
                nc.vector.tensor_tensor(
                    view, view,
                    tv_sb[:, n_rec + c0: n_rec + c0 + cw, None].to_broadcast(
                        [P, cw, TB]),
                    mybir.AluOpType.mult,
                )
                btiles[bi] = b_sb

            def emit_agg(s):
                """All message-tile matmuls of block s into one [P, DI] psum."""
                psum_s = aps.tile([P, DI], f32, tag="ap", name="apsum")
                kl, kh = plan.K[s, 0], plan.K[s, 1]
                for h, kk in ((0, kl), (1, kh)):
                    for k in range(kk):
                        st = int(plan.sbase[s, h]) + k
                        gt = st + (ntl if h else 0)
                        ensure_gchunk(h, st // CH)
                        ensure_bchunk(gt // BCH)
                        m_ap = gtiles[h][st // CH][
                            :, (st % CH) * DI: (st % CH + 1) * DI]
                        b_ap = btiles[gt // BCH][
                            :, (gt % BCH) * TB: (gt % BCH + 1) * TB]
                        nc.tensor.matmul(
                            psum_s[:, :], b_ap, m_ap,
                            start=(h == 0 and k == 0),
                            stop=(h == 1 and k == kh - 1),
                        )
                z = z_pool.tile([P, DI], bf16, tag="z", name="z_sb")
                nc.vector.tensor_copy(z[:, :], psum_s[:, :])
                return z

            ost = [None]
            base = [0]

            def emit_tail(s, z):
                """Z -> transpose -> @W -> +bias -> relu -> out batch."""
                zt = zt_pool.tile([P, DI], bf16, tag="zt", name="zt_sb")
                for k in range(cfg.kc):
                    tp = tps.tile([P, P], bf16, tag="tp", name="tpsum")
                    nc.tensor.transpose(
                        tp[:, :], z[:, k * P: (k + 1) * P], ident[:, :])
                    nc.scalar.copy(zt[:, k * P: (k + 1) * P], tp[:, :])
                opsum = ops_pool.tile([P, DO], f32, tag="op", name="opsum")
                for k in range(cfg.kc):
                    nc.tensor.matmul(
                        opsum[:, :],
                        zt[:, k * P: (k + 1) * P],
                        w_sb[:, k * DO: (k + 1) * DO],
                        start=(k == 0), stop=(k == cfg.kc - 1),
                    )
                if ost[0] is None:
                    ost[0] = ost_pool.tile([P, cfg.out_batch * DO], f32,
                                           tag="ost", name="ost")
                    base[0] = s
                osl = ost[0][:, (s - base[0]) * DO: (s - base[0] + 1) * DO]
                nc.vector.tensor_add(osl, opsum[:, :], bias_sb[:, :])
                nc.scalar.activation(osl, osl,
                                     mybir.ActivationFunctionType.Relu)
                if s - base[0] + 1 == cfg.out_batch or s == cfg.nbt - 1:
                    cnt = s - base[0] + 1
                    # NOTE: HWDGE writes to the ExternalOutput while SWDGE
                    # gathers are in flight crash the device (NRT 101);
                    # route output writes through SWDGE (gpsimd).
                    nc.gpsimd.dma_start(
                        out_r[:, base[0]: base[0] + cnt, :],
                        ost[0][:, : cnt * DO].rearrange(
                            "p (a f) -> p a f", f=DO),
                    )
                    ost[0] = None

            # software pipeline: agg(s+1) runs on PE before tail(s)
            prev = None
            for s in range(cfg.nbt):
                z = emit_agg(s)
                if prev is not None:
                    emit_tail(prev[0], prev[1])
                prev = (s, z)
            emit_tail(prev[0], prev[1])
    nc.compile()
    return nc


def _prep_shared(cfg: Cfg, x, W, b):
    xpad = np.zeros((cfg.npad, cfg.di), dtype=BF16)
    xpad[: cfg.n] = x.astype(BF16)
    x_lo = np.ascontiguousarray(xpad[: cfg.nlo])
    x_hi = np.ascontiguousarray(xpad[cfg.split:])
    w_host = np.ascontiguousarray(
        W.astype(BF16).reshape(cfg.kc, P, cfg.do).transpose(1, 0, 2).reshape(P, -1)
    )
    bias = np.ascontiguousarray(
        np.broadcast_to(b.astype(np.float32), (P, cfg.do)))
    ramp = np.ascontiguousarray(
        np.broadcast_to(np.arange(cfg.tb, dtype=np.float32), (P, cfg.tb))
    ).astype(BF16)
    return x_lo, x_hi, w_host, bias, ramp


def run(cfg: Cfg, x, edge_index, W, b, trace=False):
    plan = Plan(cfg, edge_index)
    nc = build_nc(cfg, plan)
    x_lo, x_hi, w_host, bias, ramp = _prep_shared(cfg, x, W, b)
    in_maps = []
    for c in range(cfg.cores):
        in_maps.append({
            "x_lo": x_lo, "x_hi": x_hi, "w": w_host, "bias": bias,
            "ramp": ramp,
            "gilo": plan.gidx[c][0], "gihi": plan.gidx[c][1],
            "tauval": plan.tauval[c],
        })
    res = run_bass_kernel_spmd(nc, in_maps, core_ids=list(range(cfg.cores)),
                               trace=trace)
    out = np.zeros((cfg.n, cfg.do), dtype=np.float32)
    for c in range(cfg.cores):
        o = res.results[c]["out"]
        for s, gb in enumerate(plan.core_blocks[c]):
            if gb < 0:
                continue
            t0 = gb * cfg.tb
            t1 = min(t0 + cfg.tb, cfg.n)
            out[t0:t1] = o[s * cfg.tb: s * cfg.tb + (t1 - t0)]
    return out, res


FULL = Cfg(n=50000, e=800000, di=512, do=256, cores=8)


def kernel(x, edge_index, W, b):
    out, _ = run(FULL, np.asarray(x), np.asarray(edge_index), np.asarray(W),
                 np.asarray(b))
    return out


# revision 30
# speedup vs baseline: 1.3531x; 1.0084x over previous
"""GCN layer (gnn_message_passing) on 8 Trainium2 NeuronCores.

out = relu(D^-1/2 (A+I) D^-1/2 (x @ W) + b)

Key identity: sum_msgs norm * (xW)[src] = (sum_msgs norm * x[src]) @ W.
Aggregation runs in x-space, so NO full-graph GEMM exists at all: each core
gathers raw x rows for its messages, scatter-accumulates them per 128-target
block on the PE (selection-matrix matmul), and applies W only to its own
aggregated targets (1/8 of the nodes).  Per core:

  - Target blocks of 128 nodes are load-balanced across cores (LPT on
    message count); each core owns 49 blocks (6272 padded target rows).
  - dma_gather pulls x rows (1KB, bf16) for message tiles of 128 straight
    from the x input halves (int16 index range); round-robin over all 4
    SWDGE queues keeps the 4 Q7 descriptor-gen pairs concurrent.  No
    producer dependency: gathers start at t=0.
  - B tiles (B[slot, tau] = dinv[t]*dinv[s] one-hot over the block's 128
    targets) are built on device in bulk: (ramp == tau) * val with
    broadcast tensor_tensor ops from tiny bf16 tau/val tables.
  - Per block: PSUM [128 targets, 512] accumulates B^T @ X_msgs over all
    the block's tiles (lo+hi source halves back-to-back), then
    Z -> PE-transpose -> Z^T @ W -> +bias -> ReLU -> out, software
    pipelined one block deep so the PE never waits on the fixups.

All planning is computed on the host from the integer edge list only; all
floating-point work runs on device.
"""

import ml_dtypes
import numpy as np

import concourse.bacc as bacc
import concourse.bass as bass
import concourse.mybir as mybir
import concourse.tile as tile
from concourse import library_config, masks
from concourse.bass_utils import run_bass_kernel_spmd

BF16 = ml_dtypes.bfloat16
P = 128  # partitions


class Cfg:
    def __init__(self, n, e, di, do, cores, gather_chunk=8, out_batch=17):
        self.n, self.e, self.di, self.do, self.cores = n, e, di, do, cores
        self.tb = P                       # targets per psum block
        self.gather_chunk = gather_chunk  # message tiles per dma_gather call
        self.out_batch = out_batch        # blocks per output DMA
        self.nbt_glob = (n + P - 1) // P          # global target blocks
        self.nbt = (self.nbt_glob + cores - 1) // cores  # blocks per core
        self.nt = self.nbt * P            # padded targets per core
        self.nblocks = (n + P - 1) // P   # node blocks (for x padding)
        self.npad = self.nblocks * P
        self.split_blocks = (self.nblocks + 1) // 2
        self.split = self.split_blocks * P          # lo/hi node boundary
        self.nlo = self.split
        self.nhi_pad = self.npad - self.split
        assert self.nlo <= 32768 and self.nhi_pad <= 32768
        assert di % P == 0
        self.kc = di // P                 # contraction chunks for W


class Plan:
    """Host-side integer/index preprocessing: shared structure (uniform
    across cores, baked into the program) + per-core index/tau/val data."""

    def __init__(self, cfg: Cfg, edge_index: np.ndarray):
        n, C, TB, CH = cfg.n, cfg.cores, cfg.tb, cfg.gather_chunk
        row = np.asarray(edge_index[0], dtype=np.int64)
        col = np.asarray(edge_index[1], dtype=np.int64)
        loop = np.arange(n, dtype=np.int64)
        row = np.concatenate([row, loop])
        col = np.concatenate([col, loop])
        deg = np.bincount(col, minlength=n).astype(np.float64)
        self.dinv = (1.0 / np.sqrt(deg)).astype(np.float32)

        # ---- load-balanced assignment of global target blocks to cores ----
        gblock = col // TB
        gcount = np.bincount(gblock, minlength=cfg.nbt_glob)
        order = np.argsort(-gcount, kind="stable")          # LPT order
        core_load = np.zeros(C, dtype=np.int64)
        core_nblk = np.zeros(C, dtype=np.int64)
        blk_core = np.zeros(cfg.nbt_glob, dtype=np.int64)
        for b in order:
            c = np.argmin(np.where(core_nblk < cfg.nbt, core_load, 1 << 60))
            blk_core[b] = c
            core_load[c] += gcount[b]
            core_nblk[c] += 1
        self.core_blocks = []   # core -> [global block id per slot]
        for c in range(C):
            mine = [b for b in order if blk_core[b] == c]
            mine += [-1] * (cfg.nbt - len(mine))            # pad slots
            self.core_blocks.append(np.array(mine, dtype=np.int64))

        blk_slot = np.zeros(cfg.nbt_glob, dtype=np.int64)
        for c in range(C):
            for s, b in enumerate(self.core_blocks[c]):
                if b >= 0:
                    blk_slot[b] = s

        counts = np.zeros((C, cfg.nbt, 2), dtype=np.int64)
        percore = []
        for c in range(C):
            m = blk_core[gblock] == c
            r, t = row[m], col[m]
            slot = blk_slot[gblock[m]]
            half = (r >= cfg.split).astype(np.int64)
            o = np.lexsort((r, half, slot))
            r, t, slot, half = r[o], t[o], slot[o], half[o]
            gid = slot * 2 + half
            cnt = np.bincount(gid, minlength=cfg.nbt * 2).reshape(cfg.nbt, 2)
            counts[c] = cnt
            percore.append((r, t, slot, half, gid, cnt))

        # unified tile counts per (slot, source half) across cores; >=1 so
        # every psum block is written
        self.K = np.maximum(
            np.ceil(counts.max(axis=0) / P).astype(np.int64), 1)  # [nbt,2]
        ntl = int(self.K[:, 0].sum())
        nth = int(self.K[:, 1].sum())
        self.ntiles = [ntl, nth]
        self.n_rec = ntl + nth
        self.sbase = np.stack(
            [np.concatenate([[0], np.cumsum(self.K[:, h])])[:-1] for h in (0, 1)],
            axis=1,
        )  # [nbt, 2]

        # per-core gather index streams + tau/val tables
        self.gidx = []    # (lo[128, ntl*8] i16, hi[128, nth*8])
        self.tauval = []  # [128, 2*n_rec] bf16: tau cols then val cols
        for c in range(C):
            r, t, slot, half, gid, cnt = percore[c]
            run_start = np.concatenate([[0], np.cumsum(cnt.reshape(-1))])[:-1]
            j = np.arange(len(r)) - run_start[gid]
            tile_in_run = j // P
            p_slot = j % P
            stream_tile = self.sbase[slot, half] + tile_in_run
            spos = stream_tile * P + p_slot
            gi = []
            for h in (0, 1):
                nt_h = self.ntiles[h]
                arr = np.zeros(nt_h * P, dtype=np.int64)
                used = np.zeros(nt_h * P, dtype=bool)
                m = half == h
                src = r[m] - (cfg.split if h else 0)
                arr[spos[m]] = src
                used[spos[m]] = True
                # trailing pad slots of each dma_gather call -> -1 (the Q7
                # kernel trims trailing negatives: no packets)
                if False:
                    nch = (nt_h + CH - 1) // CH
                    for ci in range(nch):
                        a, b_ = ci * CH * P, min((ci + 1) * CH, nt_h) * P
                        u = np.nonzero(used[a:b_])[0]
                        last = (u[-1] + 1) if len(u) else 0
                        arr[a + last: b_] = -1
                w = arr.astype(np.int16).reshape(-1, 16).T
                gi.append(np.tile(w, (8, 1)).copy())
            self.gidx.append(gi)

            col_of = stream_tile + np.where(half == 1, ntl, 0)
            tau = np.full((P, self.n_rec), -1.0, dtype=np.float32)
            val = np.zeros((P, self.n_rec), dtype=np.float32)
            tau[p_slot, col_of] = (t % TB).astype(np.float32)
            val[p_slot, col_of] = (self.dinv[t] * self.dinv[r]).astype(
                np.float32)
            self.tauval.append(np.ascontiguousarray(
                np.concatenate([tau, val], axis=1).astype(BF16)))


def build_nc(cfg: Cfg, plan: Plan) -> bass.Bass:
    n_rec, TB, DO, DI, CH = plan.n_rec, cfg.tb, cfg.do, cfg.di, cfg.gather_chunk
    ntl = plan.ntiles[0]
    f32, bf16, i16 = mybir.dt.float32, mybir.dt.bfloat16, mybir.dt.int16

    nc = bacc.Bacc("TRN2", target_bir_lowering=False, debug=False,
                   num_swdge_queues=4)
    x_lo = nc.dram_tensor("x_lo", [cfg.nlo, DI], bf16, kind="ExternalInput")
    x_hi = nc.dram_tensor("x_hi", [cfg.nhi_pad, DI], bf16, kind="ExternalInput")
    w = nc.dram_tensor("w", [P, cfg.kc * DO], bf16, kind="ExternalInput")
    bias = nc.dram_tensor("bias", [P, DO], f32, kind="ExternalInput")
    gilo = nc.dram_tensor("gilo", [P, plan.ntiles[0] * 8], i16, kind="ExternalInput")
    gihi = nc.dram_tensor("gihi", [P, plan.ntiles[1] * 8], i16, kind="ExternalInput")
    tauval = nc.dram_tensor("tauval", [P, 2 * n_rec], bf16, kind="ExternalInput")
    ramp = nc.dram_tensor("ramp", [P, TB], bf16, kind="ExternalInput")
    out = nc.dram_tensor("out", [cfg.nbt * P, DO], f32, kind="ExternalOutput")

    nc.gpsimd.load_library(library_config.mlp)

    with tile.TileContext(nc) as tc:
        with (
            tc.tile_pool(name="consts", bufs=1) as consts,
            tc.tile_pool(name="glo", bufs=5) as glo_pool,
            tc.tile_pool(name="ghi", bufs=5) as ghi_pool,
            tc.tile_pool(name="gidx", bufs=4) as gidx_pool,
            tc.tile_pool(name="bsel", bufs=3) as bsel_pool,
            tc.tile_pool(name="zp", bufs=2) as z_pool,
            tc.tile_pool(name="ztp", bufs=2) as zt_pool,
            tc.tile_pool(name="agg_psum", bufs=3, space="PSUM") as aps,
            tc.tile_pool(name="tp_psum", bufs=2, space="PSUM") as tps,
            tc.tile_pool(name="o_psum", bufs=2, space="PSUM") as ops_pool,
            tc.tile_pool(name="ost", bufs=2) as ost_pool,
        ):
            # ---- constants ----
            w_sb = consts.tile([P, cfg.kc * DO], bf16, tag="w")
            nc.sync.dma_start(w_sb[:], w[:, :])
            bias_sb = consts.tile([P, DO], f32, tag="bias")
            nc.sync.dma_start(bias_sb[:], bias[:, :])
            tv_sb = consts.tile([P, 2 * n_rec], bf16, tag="tauval")
            nc.sync.dma_start(tv_sb[:], tauval[:, :])
            ramp_sb = consts.tile([P, TB], bf16, tag="ramp")
            nc.sync.dma_start(ramp_sb[:], ramp[:, :])
            ident = consts.tile([P, P], bf16, tag="ident")
            masks.make_identity(nc, ident[:])

            out_r = out[:, :].rearrange("(a p) f -> p a f", p=P)
            gsrc = [x_lo, x_hi]
            gidx_dram = [gilo, gihi]
            gpools = [glo_pool, ghi_pool]
            nchunks = [(plan.ntiles[h] + CH - 1) // CH for h in (0, 1)]
            gtiles = [[None] * nchunks[0], [None] * nchunks[1]]
            gq = [0]  # rotating SWDGE queue
            IB = 16   # gather chunks of idx per idx-DMA
            gidx_tiles = [{}, {}]

            def ensure_gidx(h, bi):
                if bi in gidx_tiles[h]:
                    return
                c0 = bi * IB * CH
                cw = min(IB * CH, plan.ntiles[h] - c0)
                gi_sb = gidx_pool.tile([P, IB * CH * 8], i16, tag="gi",
                                       name="gi_sb")
                nc.sync.dma_start(gi_sb[:, : cw * 8],
                                  gidx_dram[h][:, c0 * 8: (c0 + cw) * 8])
                gidx_tiles[h][bi] = gi_sb

            def ensure_gchunk(h, ci):
                if gtiles[h][ci] is not None:
                    return
                c0 = ci * CH
                cw = min(CH, plan.ntiles[h] - c0)
                ensure_gidx(h, ci // IB)
                gi_sb = gidx_tiles[h][ci // IB][
                    :, (ci % IB) * CH * 8: (ci % IB) * CH * 8 + cw * 8]
                g_sb = gpools[h].tile([P, CH * DI], bf16, tag=f"g{h}",
                                      name="g_sb")
                nc.gpsimd.dma_gather(
                    g_sb[:, : cw * DI].rearrange("p (t f) -> p t f", f=DI),
                    gsrc[h][:, :],
                    gi_sb,
                    cw * P,
                    cw * P,
                    DI,
                    single_packet=True,
                    queue_num=gq[0],
                )
                gq[0] = (gq[0] + 1) % 4
                gtiles[h][ci] = g_sb

            # on-device B tiles in bulk chunks:
            # B[p, t, tau] = (ramp[tau] == tau[p, t]) * val[p, t]
            BCH = 32
            nbchunks = (n_rec + BCH - 1) // BCH
            btiles = [None] * nbchunks

            def ensure_bchunk(bi):
                if btiles[bi] is not None:
                    return
                c0 = bi * BCH
                cw = min(BCH, n_rec - c0)
                b_sb = bsel_pool.tile([P, BCH * TB], bf16, tag="bsel",
                                      name="b_sb")
                view = b_sb[:, : cw * TB].rearrange("p (t f) -> p t f", f=TB)
                nc.vector.tensor_tensor(
                    view,
                    ramp_sb[:, None, :].to_broadcast([P, cw, TB]),
                    tv_sb[:, c0: c0 + cw, None].to_broadcast([P, cw, TB]),
                    mybir.AluOpType.is_equal,
                )
                nc.vector.tensor_tensor(
                    view, view,
                    tv_sb[:, n_rec + c0: n_rec + c0 + cw, None].to_broadcast(
                        [P, cw, TB]),
                    mybir.AluOpType.mult,
                )
                btiles[bi] = b_sb

            def emit_agg(s):
                """All message-tile matmuls of block s into one [P, DI] psum."""
                psum_s = aps.tile([P, DI], f32, tag="ap", name="apsum")
                kl, kh = plan.K[s, 0], plan.K[s, 1]
                for h, kk in ((0, kl), (1, kh)):
                    for k in range(kk):
                        st = int(plan.sbase[s, h]) + k
                        gt = st + (ntl if h else 0)
                        ensure_gchunk(h, st // CH)
                        ensure_bchunk(gt // BCH)
                        m_ap = gtiles[h][st // CH][
                            :, (st % CH) * DI: (st % CH + 1) * DI]
                        b_ap = btiles[gt // BCH][
                            :, (gt % BCH) * TB: (gt % BCH + 1) * TB]
                        nc.tensor.matmul(
                            psum_s[:, :], b_ap, m_ap,
                            start=(h == 0 and k == 0),
                            stop=(h == 1 and k == kh - 1),
                        )
                z = z_pool.tile([P, DI], bf16, tag="z", name="z_sb")
                nc.vector.tensor_copy(z[:, :], psum_s[:, :])
                return z

            ost = [None]
            base = [0]

            def emit_tail(s, z):
                """Z -> transpose -> @W -> +bias -> relu -> out batch."""
                zt = zt_pool.tile([P, DI], bf16, tag="zt", name="zt_sb")
                for k in range(cfg.kc):
                    tp = tps.tile([P, P], bf16, tag="tp", name="tpsum")
                    nc.tensor.transpose(
                        tp[:, :], z[:, k * P: (k + 1) * P], ident[:, :])
                    nc.scalar.copy(zt[:, k * P: (k + 1) * P], tp[:, :])
                opsum = ops_pool.tile([P, DO], f32, tag="op", name="opsum")
                for k in range(cfg.kc):
                    nc.tensor.matmul(
                        opsum[:, :],
                        zt[:, k * P: (k + 1) * P],
                        w_sb[:, k * DO: (k + 1) * DO],
                        start=(k == 0), stop=(k == cfg.kc - 1),
                    )
                if ost[0] is None:
                    ost[0] = ost_pool.tile([P, cfg.out_batch * DO], f32,
                                           tag="ost", name="ost")
                    base[0] = s
                osl = ost[0][:, (s - base[0]) * DO: (s - base[0] + 1) * DO]
                nc.vector.tensor_add(osl, opsum[:, :], bias_sb[:, :])
                nc.scalar.activation(osl, osl,
                                     mybir.ActivationFunctionType.Relu)
                if s - base[0] + 1 == cfg.out_batch or s == cfg.nbt - 1:
                    cnt = s - base[0] + 1
                    # NOTE: HWDGE writes to the ExternalOutput while SWDGE
                    # gathers are in flight crash the device (NRT 101);
                    # route output writes through SWDGE (gpsimd).
                    nc.gpsimd.dma_start(
                        out_r[:, base[0]: base[0] + cnt, :],
                        ost[0][:, : cnt * DO].rearrange(
                            "p (a f) -> p a f", f=DO),
                    )
                    ost[0] = None

            # software pipeline: agg(s+1) runs on PE before tail(s)
            prev = None
            for s in range(cfg.nbt):
                z = emit_agg(s)
                if prev is not None:
                    emit_tail(prev[0], prev[1])
                prev = (s, z)
            emit_tail(prev[0], prev[1])
    nc.compile()
    return nc


def _prep_shared(cfg: Cfg, x, W, b):
    xpad = np.zeros((cfg.npad, cfg.di), dtype=BF16)
    xpad[: cfg.n] = x.astype(BF16)
    x_lo = np.ascontiguousarray(xpad[: cfg.nlo])
    x_hi = np.ascontiguousarray(xpad[cfg.split:])
    w_host = np.ascontiguousarray(
        W.astype(BF16).reshape(cfg.kc, P, cfg.do).transpose(1, 0, 2).reshape(P, -1)
    )
    bias = np.ascontiguousarray(
        np.broadcast_to(b.astype(np.float32), (P, cfg.do)))
    ramp = np.ascontiguousarray(
        np.broadcast_to(np.arange(cfg.tb, dtype=np.float32), (P, cfg.tb))
    ).astype(BF16)
    return x_lo, x_hi, w_host, bias, ramp


def run(cfg: Cfg, x, edge_index, W, b, trace=False):
    plan = Plan(cfg, edge_index)
    nc = build_nc(cfg, plan)
    x_lo, x_hi, w_host, bias, ramp = _prep_shared(cfg, x, W, b)
    in_maps = []
    for c in range(cfg.cores):
        in_maps.append({
            "x_lo": x_lo, "x_hi": x_hi, "w": w_host, "bias": bias,
            "ramp": ramp,
            "gilo": plan.gidx[c][0], "gihi": plan.gidx[c][1],
            "tauval": plan.tauval[c],
        })
    res = run_bass_kernel_spmd(nc, in_maps, core_ids=list(range(cfg.cores)),
                               trace=trace)
    out = np.zeros((cfg.n, cfg.do), dtype=np.float32)
    for c in range(cfg.cores):
        o = res.results[c]["out"]
        for s, gb in enumerate(plan.core_blocks[c]):
            if gb < 0:
                continue
            t0 = gb * cfg.tb
            t1 = min(t0 + cfg.tb, cfg.n)
            out[t0:t1] = o[s * cfg.tb: s * cfg.tb + (t1 - t0)]
    return out, res


FULL = Cfg(n=50000, e=800000, di=512, do=256, cores=8)


def kernel(x, edge_index, W, b):
    out, _ = run(FULL, np.asarray(x), np.asarray(edge_index), np.asarray(W),
                 np.asarray(b))
    return out
